# revision 1
# baseline (speedup 1.0000x reference)
"""Trainium2 Bass kernel for DirectInterpGNN message passing.

Math (per reference):
    num_v  = sum_{e: src_e=v} A_e
    den_v  = sum_{e: src_e=v} A_e*S_e*v_e
    f_v    = (C_v - 1) * (num_v/den_v) / A_ii_v
    w_e    = A_e * f_{src_e}

Distribution: edges split contiguously across 8 NeuronCores (2M edges each),
vertex table replicated. Each core computes partial per-vertex sums via
PE-deduplicated indirect scatter-add into K replicated DRAM tables, the
partials are AllReduced across the 8 cores, each core computes the per-vertex
factor f, then re-walks its edges gathering f[src] to produce w.

Per-128-edge-tile scatter correctness: indices within a tile are deduplicated
with a PE selection-matrix (duplicate edges' values are pre-summed by a
matmul and only the first occurrence row carries a real index; duplicates are
routed to a trash row). Tiles round-robin over K independent table replicas so
in-flight scatter-adds never touch the same replica concurrently (Tile
serializes same-replica writers); replicas are summed at the end.
"""
import sys
sys.path.insert(0, '/opt/trn_rl_repo')
sys.path.insert(0, '/root/.axon_site/_ro/trn_rl_repo')

import numpy as np

P = 128
N_CORES = 8

# full-size problem constants (hardcoded per task spec)
E_FULL = 16_000_000
N_VERT = 500_000


def _params(e_core, n_vert, u_tiles, n_chunks):
    cols = -(-n_vert // P // 2) * 2 + 2          # vertices per partition (even, +pad)
    while (cols * P) % (2 * n_chunks) or cols % n_chunks:
        cols += 2
    npad = cols * P
    assert e_core % (P * u_tiles) == 0
    return dict(
        E_CORE=e_core, NPAD=npad, COLS=cols, TRASH=npad - 1,
        U=u_tiles, OUTER=e_core // (P * u_tiles), NCHUNK=n_chunks,
        CHW=2 * cols // n_chunks,               # chunk width in the [P, 2*COLS] view
        FCW=cols // n_chunks,                   # chunk width in the [P, COLS] f view
    )


FULL = dict(e_core=E_FULL // N_CORES, n_vert=N_VERT, u_tiles=25, n_chunks=4)
K_REP = 16


def build_kernel(e_core, n_vert, u_tiles, n_chunks, n_cores=N_CORES,
                 use_collective=True):
    import concourse.bass as bass
    import concourse.bacc as bacc
    import concourse.mybir as mybir
    import concourse.tile as tile
    from concourse.masks import make_identity

    p = _params(e_core, n_vert, u_tiles, n_chunks)
    E_CORE, NPAD, COLS, TRASH = p["E_CORE"], p["NPAD"], p["COLS"], p["TRASH"]
    U, OUTER, NCHUNK, CHW, FCW = p["U"], p["OUTER"], p["NCHUNK"], p["CHW"], p["FCW"]
    TE = P * U                                   # edges per outer iteration
    f32 = mybir.dt.float32
    i32 = mybir.dt.int32

    nc = bacc.Bacc("TRN2", target_bir_lowering=False, debug=False,
                   num_devices=n_cores)
    src = nc.dram_tensor("src", [OUTER, P, U], i32, kind="ExternalInput")
    attr = nc.dram_tensor("attr", [OUTER, P, 3 * U], f32, kind="ExternalInput")
    vattr = nc.dram_tensor("vattr", [NPAD, 2], f32, kind="ExternalInput")
    w = nc.dram_tensor("w", [OUTER, P, U], f32, kind="ExternalOutput")

    with tile.TileContext(nc) as tc:
        with (tc.tile_pool(name="const", bufs=1) as cpool,
              tc.tile_pool(name="work", bufs=2) as wpool,
              tc.tile_pool(name="mwork", bufs=3) as mpool,
              tc.tile_pool(name="psT", bufs=3, space="PSUM") as psT_pool,
              tc.tile_pool(name="psS", bufs=2, space="PSUM") as psS_pool,
              tc.tile_pool(name="dram", bufs=1, space="DRAM") as dpool):

            ident = cpool.tile([P, P], f32)
            make_identity(nc, ident[:])
            # strict lower-triangular mask: LT[p, q] = 1 if q < p else 0
            lt = cpool.tile([P, P], f32)
            iot_q = cpool.tile([P, P], i32)
            nc.gpsimd.iota(iot_q[:], pattern=[[1, P]], base=0,
                           channel_multiplier=0)
            iot_p = cpool.tile([P, P], i32)
            nc.gpsimd.iota(iot_p[:], pattern=[[0, P]], base=0,
                           channel_multiplier=1)
            iot_qf = cpool.tile([P, P], f32)
            nc.vector.tensor_copy(iot_qf[:], iot_q[:])
            iot_pf = cpool.tile([P, P], f32)
            nc.vector.tensor_copy(iot_pf[:], iot_p[:])
            nc.vector.tensor_tensor(
                out=lt[:], in0=iot_qf[:], in1=iot_pf[:],
                op=mybir.AluOpType.is_lt)

            reps = []
            for k in range(K_REP):
                rk = dpool.tile([NPAD, 2], f32, name=f"rep{k}")
                reps.append(rk)
            zt = cpool.tile([P, CHW], f32)
            nc.vector.memset(zt[:], 0.0)
            for k in range(K_REP):
                rv = reps[k][:].rearrange("(p c) v -> p (c v)", p=P)
                for ch in range(NCHUNK):
                    nc.sync.dma_start(rv[:, ch * CHW:(ch + 1) * CHW], zt[:])

            # ---------------- phase A: dedup + scatter-add ----------------
            with tc.For_i(0, OUTER, 1) as i:
                src_t = wpool.tile([P, U], i32)
                nc.sync.dma_start(src_t[:], src[i, :, :])
                attr_t = wpool.tile([P, 3 * U], f32)
                nc.sync.dma_start(attr_t[:], attr[i, :, :])
                at3 = attr_t[:].rearrange("p (j v) -> p j v", v=3)
                a_v = at3[:, :, 0]
                s_v = at3[:, :, 1]
                v_v = at3[:, :, 2]

                idxf = wpool.tile([P, U], f32)
                nc.vector.tensor_copy(idxf[:], src_t[:])
                m_sl = wpool.tile([P, U], f32)
                nc.vector.tensor_tensor(
                    out=m_sl[:], in0=a_v, in1=s_v, op=mybir.AluOpType.mult)
                nc.vector.tensor_tensor(
                    out=m_sl[:], in0=m_sl[:], in1=v_v, op=mybir.AluOpType.mult)
                paired = wpool.tile([P, 2 * U], f32)
                pr3 = paired[:].rearrange("p (j v) -> p j v", v=2)
                nc.vector.tensor_copy(pr3[:, :, 0], a_v)
                nc.vector.tensor_copy(pr3[:, :, 1], m_sl[:])

                occ = wpool.tile([P, U], f32)
                psumS = psS_pool.tile([P, 2 * U], f32)
                for j in range(U):
                    col = idxf[:, j:j + 1]
                    psumT = psT_pool.tile([P, P], f32, tag="psT")
                    nc.tensor.transpose(
                        out=psumT[:], in_=col.to_broadcast([P, P]),
                        identity=ident[:])
                    idxT = mpool.tile([P, P], f32, tag="idxT")
                    nc.vector.tensor_copy(idxT[:], psumT[:])
                    msel = mpool.tile([P, P], f32, tag="msel")
                    nc.vector.tensor_tensor(
                        out=msel[:], in0=col.to_broadcast([P, P]), in1=idxT[:],
                        op=mybir.AluOpType.is_equal)
                    scrap = mpool.tile([P, P], f32, tag="scrap")
                    nc.vector.scalar_tensor_tensor(
                        out=scrap[:], in0=msel[:], scalar=1.0, in1=lt[:],
                        op0=mybir.AluOpType.mult, op1=mybir.AluOpType.mult,
                        accum_out=occ[:, j:j + 1])
                    nc.tensor.matmul(
                        out=psumS[:, 2 * j:2 * j + 2], lhsT=msel[:],
                        rhs=pr3[:, j, :], start=True, stop=True)

                svals = wpool.tile([P, 2 * U], f32)
                nc.vector.tensor_copy(svals[:], psumS[:])
                mask = wpool.tile([P, U], f32)
                nc.vector.tensor_scalar(
                    out=mask[:], in0=occ[:], scalar1=0.0, scalar2=None,
                    op0=mybir.AluOpType.is_equal)
                sidxf = wpool.tile([P, U], f32)
                nc.vector.scalar_tensor_tensor(
                    out=sidxf[:], in0=idxf[:], scalar=float(-TRASH), in1=mask[:],
                    op0=mybir.AluOpType.add, op1=mybir.AluOpType.mult)
                nc.vector.tensor_scalar(
                    out=sidxf[:], in0=sidxf[:], scalar1=float(TRASH), scalar2=None,
                    op0=mybir.AluOpType.add)
                sidx = wpool.tile([P, U], i32)
                nc.vector.tensor_copy(sidx[:], sidxf[:])
                sv3 = svals[:].rearrange("p (j v) -> p j v", v=2)
                for j in range(U):
                    nc.gpsimd.indirect_dma_start(
                        out=reps[j % K_REP][:],
                        out_offset=bass.IndirectOffsetOnAxis(
                            ap=sidx[:, j:j + 1], axis=0),
                        in_=sv3[:, j, :],
                        in_offset=None,
                        compute_op=mybir.AluOpType.add)

            # ---------------- merge replicas ----------------
            partial = dpool.tile([P, 2 * COLS], f32)
            for ch in range(NCHUNK):
                sl = slice(ch * CHW, (ch + 1) * CHW)
                acc = mpool.tile([P, CHW], f32, tag="acc")
                nc.sync.dma_start(
                    acc[:], reps[0][:].rearrange("(p c) v -> p (c v)", p=P)[:, sl])
                for k in range(1, K_REP):
                    tk = mpool.tile([P, CHW], f32, tag="tk")
                    nc.sync.dma_start(
                        tk[:],
                        reps[k][:].rearrange("(p c) v -> p (c v)", p=P)[:, sl])
                    nc.vector.tensor_tensor(
                        out=acc[:], in0=acc[:], in1=tk[:],
                        op=mybir.AluOpType.add)
                nc.sync.dma_start(partial[:, sl], acc[:])

            # ---------------- all-reduce ----------------
            if use_collective:
                ar_out = dpool.tile([P, 2 * COLS], f32, name="ar_out")
                nc.gpsimd.collective_compute(
                    "AllReduce", mybir.AluOpType.add,
                    replica_groups=[list(range(n_cores))],
                    ins=[partial.opt()],
                    outs=[ar_out.opt()])
                table = ar_out
            else:
                table = partial

            # ---------------- vertex math: f = (C-1)*num/den/A_ii ----------
            f_tab = dpool.tile([NPAD, 1], f32)
            fv = f_tab[:].rearrange("(p c) v -> p (c v)", p=P)
            for ch in range(NCHUNK):
                sl = slice(ch * CHW, (ch + 1) * CHW)
                tt = mpool.tile([P, CHW], f32, tag="tt")
                nc.sync.dma_start(tt[:], table[:, sl])
                va = mpool.tile([P, CHW], f32, tag="va")
                nc.sync.dma_start(
                    va[:], vattr[:].rearrange("(p c) v -> p (c v)", p=P)[:, sl])
                tt3 = tt[:].rearrange("p (c v) -> p c v", v=2)
                va3 = va[:].rearrange("p (c v) -> p c v", v=2)
                fch = mpool.tile([P, FCW], f32, tag="fch")
                dsafe = mpool.tile([P, FCW], f32, tag="dsafe")
                # den==0 only for vertices with no incident edges (num==0 too,
                # so f becomes 0 instead of NaN)
                nc.vector.tensor_scalar(
                    out=dsafe[:], in0=tt3[:, :, 1], scalar1=0.0, scalar2=None,
                    op0=mybir.AluOpType.is_equal)
                nc.vector.tensor_tensor(
                    out=dsafe[:], in0=dsafe[:], in1=tt3[:, :, 1],
                    op=mybir.AluOpType.add)
                # fold A_ii into the denominator, then one reciprocal
                nc.vector.tensor_tensor(
                    out=dsafe[:], in0=dsafe[:], in1=va3[:, :, 0],
                    op=mybir.AluOpType.mult)
                nc.vector.reciprocal(out=dsafe[:], in_=dsafe[:])
                nc.vector.tensor_tensor(
                    out=fch[:], in0=tt3[:, :, 0], in1=dsafe[:],
                    op=mybir.AluOpType.mult)
                cm1 = mpool.tile([P, FCW], f32, tag="cm1")
                nc.vector.tensor_scalar(
                    out=cm1[:], in0=va3[:, :, 1], scalar1=-1.0, scalar2=None,
                    op0=mybir.AluOpType.add)
                nc.vector.tensor_tensor(
                    out=fch[:], in0=fch[:], in1=cm1[:],
                    op=mybir.AluOpType.mult)
                nc.sync.dma_start(fv[:, ch * FCW:(ch + 1) * FCW], fch[:])

            # ---------------- phase C: w = A * f[src] ----------------
            with tc.For_i(0, OUTER, 1) as i:
                src_t2 = wpool.tile([P, U], i32)
                nc.sync.dma_start(src_t2[:], src[i, :, :])
                attr_t2 = wpool.tile([P, 3 * U], f32)
                nc.sync.dma_start(attr_t2[:], attr[i, :, :])
                fg = wpool.tile([P, U], f32)
                for j in range(U):
                    nc.gpsimd.indirect_dma_start(
                        out=fg[:, j:j + 1],
                        out_offset=None,
                        in_=f_tab[:],
                        in_offset=bass.IndirectOffsetOnAxis(
                            ap=src_t2[:, j:j + 1], axis=0))
                wt = wpool.tile([P, U], f32)
                nc.vector.tensor_tensor(
                    out=wt[:],
                    in0=attr_t2[:].rearrange("p (j v) -> p j v", v=3)[:, :, 0],
                    in1=fg[:], op=mybir.AluOpType.mult)
                nc.sync.dma_start(w[i, :, :], wt[:])

    nc.compile()
    return nc, p


_CACHE = {}


def _get_full_kernel():
    key = "full"
    if key not in _CACHE:
        _CACHE[key] = build_kernel(**FULL)
    return _CACHE[key]


def kernel(vertex_attr, edge_attr, edgeij_pair):
    from concourse.bass_utils import run_bass_kernel_spmd

    nc, p = _get_full_kernel()
    NPAD = p["NPAD"]
    E_CORE = p["E_CORE"]

    vertex_attr = np.asarray(vertex_attr, dtype=np.float32)
    edge_attr = np.ascontiguousarray(np.asarray(edge_attr, dtype=np.float32))
    srcf = np.ascontiguousarray(np.asarray(edgeij_pair, dtype=np.int32)[0])

    vpad = np.ones((NPAD, 2), dtype=np.float32)
    vpad[:vertex_attr.shape[0]] = vertex_attr

    in_maps = []
    for c in range(N_CORES):
        sl = slice(c * E_CORE, (c + 1) * E_CORE)
        outer = E_CORE // (P * FULL["u_tiles"])
        in_maps.append({
            "src": srcf[sl].reshape(outer, 128, FULL["u_tiles"]),
            "attr": edge_attr[sl].reshape(outer, 128, 3 * FULL["u_tiles"]),
            "vattr": vpad,
        })
    res = run_bass_kernel_spmd(nc, in_maps, list(range(N_CORES)))
    return np.concatenate(
        [res.results[c]["w"].reshape(-1) for c in range(N_CORES)])



# revision 3
# speedup vs baseline: 1.2768x; 1.2768x over previous
"""Trainium2 Bass kernel for DirectInterpGNN message passing (optimized).

Edges sharded contiguously across 8 cores; vertex table replicated; per-core
partial segment sums via PE-deduplicated indirect scatter-add into 16 DRAM
replica tables; AllReduce; per-vertex factor f; per-edge gather of f[src].

Key structure vs v2/baseline:
- deeper tile-pool buffering so the gpsimd SWDGE stream never stalls
- dedup trimmed: is_equal reads the PE transpose directly from PSUM
- src + A columns stashed in SBUF during phase A; phase C does no HBM loads
- phase C gathers 25 columns into one [128,25] tile (subtile deps), one multiply
"""
import sys
sys.path.insert(0, '/opt/trn_rl_repo')
sys.path.insert(0, '/root/.axon_site/_ro/trn_rl_repo')

import numpy as np

P = 128
N_CORES = 8
E_FULL = 16_000_000
N_VERT = 500_000


def _params(e_core, n_vert, u_tiles, n_chunks):
    cols = -(-n_vert // P // 2) * 2 + 2
    while (cols * P) % (2 * n_chunks) or cols % n_chunks:
        cols += 2
    npad = cols * P
    assert e_core % (P * u_tiles) == 0
    return dict(
        E_CORE=e_core, NPAD=npad, COLS=cols, TRASH=npad - 1,
        U=u_tiles, OUTER=e_core // (P * u_tiles), NCHUNK=n_chunks,
        CHW=2 * cols // n_chunks,
        FCW=cols // n_chunks,
    )


FULL = dict(e_core=E_FULL // N_CORES, n_vert=N_VERT, u_tiles=25, n_chunks=8)


def build_kernel(e_core=FULL["e_core"], n_vert=FULL["n_vert"],
                 u_tiles=FULL["u_tiles"], n_chunks=FULL["n_chunks"],
                 n_cores=N_CORES, k_rep=16, wb=4, mb=4, fgb=8, ua=4, uc=8,
                 phase_a=True, phase_c=True,
                 use_collective=True):
    import concourse.bass as bass
    import concourse.bacc as bacc
    import concourse.mybir as mybir
    import concourse.tile as tile
    from concourse.masks import make_identity

    p = _params(e_core, n_vert, u_tiles, n_chunks)
    E_CORE, NPAD, COLS, TRASH = p["E_CORE"], p["NPAD"], p["COLS"], p["TRASH"]
    U, OUTER, NCHUNK, CHW, FCW = p["U"], p["OUTER"], p["NCHUNK"], p["CHW"], p["FCW"]
    f32 = mybir.dt.float32
    i32 = mybir.dt.int32

    nc = bacc.Bacc("TRN2", target_bir_lowering=False, debug=False,
                   num_devices=n_cores)
    src = nc.dram_tensor("src", [OUTER, P, U], i32, kind="ExternalInput")
    attr = nc.dram_tensor("attr", [OUTER, P, 3 * U], f32, kind="ExternalInput")
    vattr = nc.dram_tensor("vattr", [NPAD, 2], f32, kind="ExternalInput")
    w = nc.dram_tensor("w", [OUTER, P, U], f32, kind="ExternalOutput")

    with tile.TileContext(nc) as tc:
        with (tc.tile_pool(name="const", bufs=1) as cpool,
              tc.tile_pool(name="work", bufs=wb) as wpool,
              tc.tile_pool(name="mwork", bufs=mb) as mpool,
              tc.tile_pool(name="chunk", bufs=2) as kpool,
              tc.tile_pool(name="fg", bufs=fgb) as fgpool,
              tc.tile_pool(name="sv", bufs=8) as svpool,
              tc.tile_pool(name="psT", bufs=4, space="PSUM") as psT_pool,
              tc.tile_pool(name="psS", bufs=4, space="PSUM") as psS_pool,
              tc.tile_pool(name="dram", bufs=1, space="DRAM") as dpool):

            stash_src = cpool.tile([P, OUTER * U], i32)
            stash_a = cpool.tile([P, OUTER * U], f32)
            ss3 = stash_src[:].rearrange("p (o u) -> p o u", u=U)
            sa3 = stash_a[:].rearrange("p (o u) -> p o u", u=U)

            ident = cpool.tile([P, P], f32)
            make_identity(nc, ident[:])
            lt = cpool.tile([P, P], f32)
            iot_q = cpool.tile([P, P], i32)
            nc.gpsimd.iota(iot_q[:], pattern=[[1, P]], base=0,
                           channel_multiplier=0)
            iot_p = cpool.tile([P, P], i32)
            nc.gpsimd.iota(iot_p[:], pattern=[[0, P]], base=0,
                           channel_multiplier=1)
            iot_qf = cpool.tile([P, P], f32)
            nc.vector.tensor_copy(iot_qf[:], iot_q[:])
            iot_pf = cpool.tile([P, P], f32)
            nc.vector.tensor_copy(iot_pf[:], iot_p[:])
            nc.vector.tensor_tensor(
                out=lt[:], in0=iot_qf[:], in1=iot_pf[:],
                op=mybir.AluOpType.is_lt)

            reps = []
            for k in range(k_rep):
                rk = dpool.tile([NPAD, 2], f32, name=f"rep{k}")
                reps.append(rk)
            zt = cpool.tile([P, CHW], f32)
            nc.vector.memset(zt[:], 0.0)
            for k in range(k_rep):
                rv = reps[k][:].rearrange("(p c) v -> p (c v)", p=P)
                for ch in range(NCHUNK):
                    nc.sync.dma_start(rv[:, ch * CHW:(ch + 1) * CHW], zt[:])

            # ---------------- phase A: dedup + scatter-add ----------------
            def body_a(i):
                src_t = wpool.tile([P, U], i32)
                nc.sync.dma_start(src_t[:], src[i, :, :])
                attr_t = wpool.tile([P, 3 * U], f32)
                nc.sync.dma_start(attr_t[:], attr[i, :, :])
                at3 = attr_t[:].rearrange("p (j v) -> p j v", v=3)
                a_v = at3[:, :, 0]
                s_v = at3[:, :, 1]
                v_v = at3[:, :, 2]

                nc.vector.tensor_copy(ss3[:, i, :], src_t[:])
                nc.vector.tensor_copy(sa3[:, i, :], a_v)

                paired = wpool.tile([P, 2 * U], f32)
                pr3 = paired[:].rearrange("p (j v) -> p j v", v=2)
                nc.vector.tensor_copy(pr3[:, :, 0], a_v)
                nc.vector.tensor_tensor(
                    out=pr3[:, :, 1], in0=a_v, in1=s_v,
                    op=mybir.AluOpType.mult)
                nc.vector.tensor_tensor(
                    out=pr3[:, :, 1], in0=pr3[:, :, 1], in1=v_v,
                    op=mybir.AluOpType.mult)

                idxf = wpool.tile([P, U], f32)
                nc.vector.tensor_copy(idxf[:], src_t[:])
                occ = wpool.tile([P, U], f32)
                psumS = psS_pool.tile([P, 2 * U], f32)
                for j in range(U):
                    col = idxf[:, j:j + 1]
                    psumT = psT_pool.tile([P, P], f32, tag="psT")
                    nc.tensor.transpose(
                        out=psumT[:], in_=col.to_broadcast([P, P]),
                        identity=ident[:])
                    msel = mpool.tile([P, P], f32, tag="msel")
                    nc.vector.tensor_tensor(
                        out=msel[:], in0=col.to_broadcast([P, P]),
                        in1=psumT[:],
                        op=mybir.AluOpType.is_equal)
                    scrap = mpool.tile([P, P], f32, tag="scrap")
                    nc.vector.scalar_tensor_tensor(
                        out=scrap[:], in0=msel[:], scalar=1.0, in1=lt[:],
                        op0=mybir.AluOpType.mult,
                        op1=mybir.AluOpType.mult,
                        accum_out=occ[:, j:j + 1])
                    nc.tensor.matmul(
                        out=psumS[:, 2 * j:2 * j + 2], lhsT=msel[:],
                        rhs=pr3[:, j, :], start=True, stop=True)

                svals = svpool.tile([P, 2 * U], f32, tag="svals")
                nc.vector.tensor_copy(svals[:], psumS[:])
                mask = wpool.tile([P, U], f32)
                nc.vector.tensor_scalar(
                    out=mask[:], in0=occ[:], scalar1=0.0, scalar2=None,
                    op0=mybir.AluOpType.is_equal)
                sidxf = wpool.tile([P, U], f32)
                nc.vector.scalar_tensor_tensor(
                    out=sidxf[:], in0=idxf[:], scalar=float(-TRASH),
                    in1=mask[:],
                    op0=mybir.AluOpType.add, op1=mybir.AluOpType.mult)
                nc.vector.tensor_scalar(
                    out=sidxf[:], in0=sidxf[:], scalar1=float(TRASH),
                    scalar2=None,
                    op0=mybir.AluOpType.add)
                sidx = svpool.tile([P, U], i32, tag="sidx")
                nc.vector.tensor_copy(sidx[:], sidxf[:])
                sv3 = svals[:].rearrange("p (j v) -> p j v", v=2)
                for j in range(U):
                    nc.gpsimd.indirect_dma_start(
                        out=reps[j % k_rep][:],
                        out_offset=bass.IndirectOffsetOnAxis(
                            ap=sidx[:, j:j + 1], axis=0),
                        in_=sv3[:, j, :],
                        in_offset=None,
                        compute_op=mybir.AluOpType.add)

            if phase_a:
                tc.For_i_unrolled(0, OUTER, 1, body_a, max_unroll=ua)

            # ---------------- merge replicas ----------------
            partial = dpool.tile([P, 2 * COLS], f32)
            for ch in range(NCHUNK):
                sl = slice(ch * CHW, (ch + 1) * CHW)
                acc = kpool.tile([P, CHW], f32, tag="acc")
                nc.sync.dma_start(
                    acc[:],
                    reps[0][:].rearrange("(p c) v -> p (c v)", p=P)[:, sl])
                for k in range(1, k_rep):
                    tk = kpool.tile([P, CHW], f32, tag="tk")
                    nc.sync.dma_start(
                        tk[:],
                        reps[k][:].rearrange("(p c) v -> p (c v)", p=P)[:, sl])
                    nc.vector.tensor_tensor(
                        out=acc[:], in0=acc[:], in1=tk[:],
                        op=mybir.AluOpType.add)
                nc.sync.dma_start(partial[:, sl], acc[:])

            # ---------------- all-reduce ----------------
            if use_collective:
                ar_out = dpool.tile([P, 2 * COLS], f32, name="ar_out")
                nc.gpsimd.collective_compute(
                    "AllReduce", mybir.AluOpType.add,
                    replica_groups=[list(range(n_cores))],
                    ins=[partial.opt()],
                    outs=[ar_out.opt()])
                table = ar_out
            else:
                table = partial

            # ---------------- vertex math: f = (C-1)*num/den/A_ii ----------
            f_tab = dpool.tile([NPAD, 1], f32)
            fv = f_tab[:].rearrange("(p c) v -> p (c v)", p=P)
            for ch in range(NCHUNK):
                sl = slice(ch * CHW, (ch + 1) * CHW)
                tt = kpool.tile([P, CHW], f32, tag="tt")
                nc.sync.dma_start(tt[:], table[:, sl])
                va = kpool.tile([P, CHW], f32, tag="va")
                nc.sync.dma_start(
                    va[:],
                    vattr[:].rearrange("(p c) v -> p (c v)", p=P)[:, sl])
                tt3 = tt[:].rearrange("p (c v) -> p c v", v=2)
                va3 = va[:].rearrange("p (c v) -> p c v", v=2)
                fch = kpool.tile([P, FCW], f32, tag="fch")
                dsafe = kpool.tile([P, FCW], f32, tag="dsafe")
                nc.vector.tensor_scalar(
                    out=dsafe[:], in0=tt3[:, :, 1], scalar1=0.0, scalar2=None,
                    op0=mybir.AluOpType.is_equal)
                nc.vector.tensor_tensor(
                    out=dsafe[:], in0=dsafe[:], in1=tt3[:, :, 1],
                    op=mybir.AluOpType.add)
                nc.vector.tensor_tensor(
                    out=dsafe[:], in0=dsafe[:], in1=va3[:, :, 0],
                    op=mybir.AluOpType.mult)
                nc.vector.reciprocal(out=dsafe[:], in_=dsafe[:])
                nc.vector.tensor_tensor(
                    out=fch[:], in0=tt3[:, :, 0], in1=dsafe[:],
                    op=mybir.AluOpType.mult)
                cm1 = kpool.tile([P, FCW], f32, tag="cm1")
                nc.vector.tensor_scalar(
                    out=cm1[:], in0=va3[:, :, 1], scalar1=-1.0, scalar2=None,
                    op0=mybir.AluOpType.add)
                nc.vector.tensor_tensor(
                    out=fch[:], in0=fch[:], in1=cm1[:],
                    op=mybir.AluOpType.mult)
                nc.sync.dma_start(fv[:, ch * FCW:(ch + 1) * FCW], fch[:])

            # ---------------- phase C: w = A * f[src] (no HBM loads) -------
            def body_c(i):
                src_off = fgpool.tile([P, U], i32, tag="soff")
                nc.vector.tensor_copy(src_off[:], ss3[:, i, :])
                fg = fgpool.tile([P, U], f32, tag="fg")
                for j in range(U):
                    nc.gpsimd.indirect_dma_start(
                        out=fg[:, j:j + 1],
                        out_offset=None,
                        in_=f_tab[:],
                        in_offset=bass.IndirectOffsetOnAxis(
                            ap=src_off[:, j:j + 1], axis=0))
                wt = fgpool.tile([P, U], f32, tag="wt")
                nc.vector.tensor_tensor(
                    out=wt[:], in0=sa3[:, i, :], in1=fg[:],
                    op=mybir.AluOpType.mult)
                nc.sync.dma_start(w[i, :, :], wt[:])

            if phase_c:
                tc.For_i_unrolled(0, OUTER, 1, body_c, max_unroll=uc)

    nc.compile()
    return nc, p


_CACHE = {}


def _get_full_kernel():
    if "full" not in _CACHE:
        _CACHE["full"] = build_kernel()
    return _CACHE["full"]


def kernel(vertex_attr, edge_attr, edgeij_pair):
    from concourse.bass_utils import run_bass_kernel_spmd

    nc, p = _get_full_kernel()
    NPAD = p["NPAD"]
    E_CORE = p["E_CORE"]
    U = p["U"]

    vertex_attr = np.asarray(vertex_attr, dtype=np.float32)
    edge_attr = np.ascontiguousarray(np.asarray(edge_attr, dtype=np.float32))
    srcf = np.ascontiguousarray(np.asarray(edgeij_pair, dtype=np.int32)[0])

    vpad = np.ones((NPAD, 2), dtype=np.float32)
    vpad[:vertex_attr.shape[0]] = vertex_attr

    in_maps = []
    for c in range(N_CORES):
        sl = slice(c * E_CORE, (c + 1) * E_CORE)
        outer = E_CORE // (P * U)
        in_maps.append({
            "src": srcf[sl].reshape(outer, P, U),
            "attr": edge_attr[sl].reshape(outer, P, 3 * U),
            "vattr": vpad,
        })
    res = run_bass_kernel_spmd(nc, in_maps, list(range(N_CORES)))
    return np.concatenate(
        [res.results[c]["w"].reshape(-1) for c in range(N_CORES)])


# revision 4
# speedup vs baseline: 1.2937x; 1.0132x over previous
"""Trainium2 Bass kernel for DirectInterpGNN message passing (optimized).

Edges sharded contiguously across 8 cores; vertex table replicated; per-core
partial segment sums via PE-deduplicated indirect scatter-add into 16 DRAM
replica tables; AllReduce; per-vertex factor f; per-edge gather of f[src].

Key structure vs v2/baseline:
- deeper tile-pool buffering so the gpsimd SWDGE stream never stalls
- dedup trimmed: is_equal reads the PE transpose directly from PSUM
- src + A columns stashed in SBUF during phase A; phase C does no HBM loads
- phase C gathers 25 columns into one [128,25] tile (subtile deps), one multiply
"""
import sys
sys.path.insert(0, '/opt/trn_rl_repo')
sys.path.insert(0, '/root/.axon_site/_ro/trn_rl_repo')

import numpy as np

P = 128
N_CORES = 8
E_FULL = 16_000_000
N_VERT = 500_000


def _params(e_core, n_vert, u_tiles, n_chunks):
    cols = -(-n_vert // P // 2) * 2 + 2
    while (cols * P) % (2 * n_chunks) or cols % n_chunks:
        cols += 2
    npad = cols * P
    assert e_core % (P * u_tiles) == 0
    return dict(
        E_CORE=e_core, NPAD=npad, COLS=cols, TRASH=npad - 1,
        U=u_tiles, OUTER=e_core // (P * u_tiles), NCHUNK=n_chunks,
        CHW=2 * cols // n_chunks,
        FCW=cols // n_chunks,
    )


FULL = dict(e_core=E_FULL // N_CORES, n_vert=N_VERT, u_tiles=25, n_chunks=8)


def build_kernel(e_core=FULL["e_core"], n_vert=FULL["n_vert"],
                 u_tiles=FULL["u_tiles"], n_chunks=FULL["n_chunks"],
                 n_cores=N_CORES, k_rep=16, wb=4, mb=4, fgb=8, ua=4, uc=8,
                 phase_a=True, phase_c=True,
                 use_collective=True):
    import concourse.bass as bass
    import concourse.bacc as bacc
    import concourse.mybir as mybir
    import concourse.tile as tile
    from concourse.masks import make_identity

    p = _params(e_core, n_vert, u_tiles, n_chunks)
    E_CORE, NPAD, COLS, TRASH = p["E_CORE"], p["NPAD"], p["COLS"], p["TRASH"]
    U, OUTER, NCHUNK, CHW, FCW = p["U"], p["OUTER"], p["NCHUNK"], p["CHW"], p["FCW"]
    f32 = mybir.dt.float32
    i32 = mybir.dt.int32

    nc = bacc.Bacc("TRN2", target_bir_lowering=False, debug=False,
                   num_devices=n_cores)
    src = nc.dram_tensor("src", [OUTER, P, U], i32, kind="ExternalInput")
    attr = nc.dram_tensor("attr", [OUTER, P, 3 * U], f32, kind="ExternalInput")
    vattr = nc.dram_tensor("vattr", [NPAD, 2], f32, kind="ExternalInput")
    w = nc.dram_tensor("w", [OUTER, P, U], f32, kind="ExternalOutput")

    with tile.TileContext(nc) as tc:
        with (tc.tile_pool(name="const", bufs=1) as cpool,
              tc.tile_pool(name="work", bufs=wb) as wpool,
              tc.tile_pool(name="mwork", bufs=mb) as mpool,
              tc.tile_pool(name="chunk", bufs=2) as kpool,
              tc.tile_pool(name="fg", bufs=fgb) as fgpool,
              tc.tile_pool(name="sv", bufs=8) as svpool,
              tc.tile_pool(name="psT", bufs=4, space="PSUM") as psT_pool,
              tc.tile_pool(name="psS", bufs=4, space="PSUM") as psS_pool,
              tc.tile_pool(name="dram", bufs=1, space="DRAM") as dpool):

            stash_src = cpool.tile([P, OUTER * U], i32)
            stash_a = cpool.tile([P, OUTER * U], f32)
            ss3 = stash_src[:].rearrange("p (o u) -> p o u", u=U)
            sa3 = stash_a[:].rearrange("p (o u) -> p o u", u=U)

            ident = cpool.tile([P, P], f32)
            make_identity(nc, ident[:])
            lt = cpool.tile([P, P], f32)
            iot_q = cpool.tile([P, P], i32)
            nc.gpsimd.iota(iot_q[:], pattern=[[1, P]], base=0,
                           channel_multiplier=0)
            iot_p = cpool.tile([P, P], i32)
            nc.gpsimd.iota(iot_p[:], pattern=[[0, P]], base=0,
                           channel_multiplier=1)
            iot_qf = cpool.tile([P, P], f32)
            nc.vector.tensor_copy(iot_qf[:], iot_q[:])
            iot_pf = cpool.tile([P, P], f32)
            nc.vector.tensor_copy(iot_pf[:], iot_p[:])
            nc.vector.tensor_tensor(
                out=lt[:], in0=iot_qf[:], in1=iot_pf[:],
                op=mybir.AluOpType.is_lt)

            reps = []
            for k in range(k_rep):
                rk = dpool.tile([NPAD, 2], f32, name=f"rep{k}")
                reps.append(rk)
            zt = cpool.tile([P, CHW], f32)
            nc.vector.memset(zt[:], 0.0)
            for k in range(k_rep):
                rv = reps[k][:].rearrange("(p c) v -> p (c v)", p=P)
                for ch in range(NCHUNK):
                    nc.sync.dma_start(rv[:, ch * CHW:(ch + 1) * CHW], zt[:])

            # ---------------- phase A: dedup + scatter-add ----------------
            def body_a(i):
                src_t = wpool.tile([P, U], i32)
                nc.sync.dma_start(src_t[:], src[i, :, :])
                attr_t = wpool.tile([P, 3 * U], f32)
                nc.sync.dma_start(attr_t[:], attr[i, :, :])
                at3 = attr_t[:].rearrange("p (j v) -> p j v", v=3)
                a_v = at3[:, :, 0]
                s_v = at3[:, :, 1]
                v_v = at3[:, :, 2]

                nc.vector.tensor_copy(ss3[:, i, :], src_t[:])
                nc.vector.tensor_copy(sa3[:, i, :], a_v)

                paired = wpool.tile([P, 2 * U], f32)
                pr3 = paired[:].rearrange("p (j v) -> p j v", v=2)
                nc.vector.tensor_copy(pr3[:, :, 0], a_v)
                nc.vector.tensor_tensor(
                    out=pr3[:, :, 1], in0=a_v, in1=s_v,
                    op=mybir.AluOpType.mult)
                nc.vector.tensor_tensor(
                    out=pr3[:, :, 1], in0=pr3[:, :, 1], in1=v_v,
                    op=mybir.AluOpType.mult)

                idxf = wpool.tile([P, U], f32)
                nc.vector.tensor_copy(idxf[:], src_t[:])
                occ = wpool.tile([P, U], f32)
                psumS = psS_pool.tile([P, 2 * U], f32)
                for j in range(U):
                    col = idxf[:, j:j + 1]
                    psumT = psT_pool.tile([P, P], f32, tag="psT")
                    nc.tensor.transpose(
                        out=psumT[:], in_=col.to_broadcast([P, P]),
                        identity=ident[:])
                    msel = mpool.tile([P, P], f32, tag="msel")
                    nc.vector.tensor_tensor(
                        out=msel[:], in0=col.to_broadcast([P, P]),
                        in1=psumT[:],
                        op=mybir.AluOpType.is_equal)
                    scrap = mpool.tile([P, P], f32, tag="scrap")
                    nc.vector.scalar_tensor_tensor(
                        out=scrap[:], in0=msel[:], scalar=1.0, in1=lt[:],
                        op0=mybir.AluOpType.mult,
                        op1=mybir.AluOpType.mult,
                        accum_out=occ[:, j:j + 1])
                    nc.tensor.matmul(
                        out=psumS[:, 2 * j:2 * j + 2], lhsT=msel[:],
                        rhs=pr3[:, j, :], start=True, stop=True)

                svals = svpool.tile([P, 2 * U], f32, tag="svals")
                nc.vector.tensor_copy(svals[:], psumS[:])
                mask = wpool.tile([P, U], f32)
                nc.vector.tensor_scalar(
                    out=mask[:], in0=occ[:], scalar1=0.0, scalar2=None,
                    op0=mybir.AluOpType.is_equal)
                sidxf = wpool.tile([P, U], f32)
                nc.vector.scalar_tensor_tensor(
                    out=sidxf[:], in0=idxf[:], scalar=float(-TRASH),
                    in1=mask[:],
                    op0=mybir.AluOpType.add, op1=mybir.AluOpType.mult)
                nc.vector.tensor_scalar(
                    out=sidxf[:], in0=sidxf[:], scalar1=float(TRASH),
                    scalar2=None,
                    op0=mybir.AluOpType.add)
                sidx = wpool.tile([P, U], i32)
                nc.vector.tensor_copy(sidx[:], sidxf[:])
                sv3 = svals[:].rearrange("p (j v) -> p j v", v=2)
                for j in range(U):
                    nc.gpsimd.indirect_dma_start(
                        out=reps[j % k_rep][:],
                        out_offset=bass.IndirectOffsetOnAxis(
                            ap=sidx[:, j:j + 1], axis=0),
                        in_=sv3[:, j, :],
                        in_offset=None,
                        compute_op=mybir.AluOpType.add)

            if phase_a:
                tc.For_i_unrolled(0, OUTER, 1, body_a, max_unroll=ua)

            # ---------------- merge replicas ----------------
            partial = dpool.tile([P, 2 * COLS], f32)
            for ch in range(NCHUNK):
                sl = slice(ch * CHW, (ch + 1) * CHW)
                acc = kpool.tile([P, CHW], f32, tag="acc")
                nc.sync.dma_start(
                    acc[:],
                    reps[0][:].rearrange("(p c) v -> p (c v)", p=P)[:, sl])
                for k in range(1, k_rep):
                    tk = kpool.tile([P, CHW], f32, tag="tk")
                    nc.sync.dma_start(
                        tk[:],
                        reps[k][:].rearrange("(p c) v -> p (c v)", p=P)[:, sl])
                    nc.vector.tensor_tensor(
                        out=acc[:], in0=acc[:], in1=tk[:],
                        op=mybir.AluOpType.add)
                nc.sync.dma_start(partial[:, sl], acc[:])

            # ---------------- all-reduce ----------------
            if use_collective:
                ar_out = dpool.tile([P, 2 * COLS], f32, name="ar_out")
                nc.gpsimd.collective_compute(
                    "AllReduce", mybir.AluOpType.add,
                    replica_groups=[list(range(n_cores))],
                    ins=[partial.opt()],
                    outs=[ar_out.opt()])
                table = ar_out
            else:
                table = partial

            # ---------------- vertex math: f = (C-1)*num/den/A_ii ----------
            f_tab = dpool.tile([NPAD, 1], f32)
            fv = f_tab[:].rearrange("(p c) v -> p (c v)", p=P)
            for ch in range(NCHUNK):
                sl = slice(ch * CHW, (ch + 1) * CHW)
                tt = kpool.tile([P, CHW], f32, tag="tt")
                nc.sync.dma_start(tt[:], table[:, sl])
                va = kpool.tile([P, CHW], f32, tag="va")
                nc.sync.dma_start(
                    va[:],
                    vattr[:].rearrange("(p c) v -> p (c v)", p=P)[:, sl])
                tt3 = tt[:].rearrange("p (c v) -> p c v", v=2)
                va3 = va[:].rearrange("p (c v) -> p c v", v=2)
                fch = kpool.tile([P, FCW], f32, tag="fch")
                dsafe = kpool.tile([P, FCW], f32, tag="dsafe")
                nc.vector.tensor_scalar(
                    out=dsafe[:], in0=tt3[:, :, 1], scalar1=0.0, scalar2=None,
                    op0=mybir.AluOpType.is_equal)
                nc.vector.tensor_tensor(
                    out=dsafe[:], in0=dsafe[:], in1=tt3[:, :, 1],
                    op=mybir.AluOpType.add)
                nc.vector.tensor_tensor(
                    out=dsafe[:], in0=dsafe[:], in1=va3[:, :, 0],
                    op=mybir.AluOpType.mult)
                nc.vector.reciprocal(out=dsafe[:], in_=dsafe[:])
                nc.vector.tensor_tensor(
                    out=fch[:], in0=tt3[:, :, 0], in1=dsafe[:],
                    op=mybir.AluOpType.mult)
                cm1 = kpool.tile([P, FCW], f32, tag="cm1")
                nc.vector.tensor_scalar(
                    out=cm1[:], in0=va3[:, :, 1], scalar1=-1.0, scalar2=None,
                    op0=mybir.AluOpType.add)
                nc.vector.tensor_tensor(
                    out=fch[:], in0=fch[:], in1=cm1[:],
                    op=mybir.AluOpType.mult)
                nc.sync.dma_start(fv[:, ch * FCW:(ch + 1) * FCW], fch[:])

            # ---------------- phase C: w = A * f[src] (no HBM loads) -------
            def body_c(i):
                src_off = fgpool.tile([P, U], i32, tag="soff")
                nc.vector.tensor_copy(src_off[:], ss3[:, i, :])
                fg = fgpool.tile([P, U], f32, tag="fg")
                for j in range(U):
                    nc.gpsimd.indirect_dma_start(
                        out=fg[:, j:j + 1],
                        out_offset=None,
                        in_=f_tab[:],
                        in_offset=bass.IndirectOffsetOnAxis(
                            ap=src_off[:, j:j + 1], axis=0))
                wt = fgpool.tile([P, U], f32, tag="wt")
                nc.vector.tensor_tensor(
                    out=wt[:], in0=sa3[:, i, :], in1=fg[:],
                    op=mybir.AluOpType.mult)
                nc.sync.dma_start(w[i, :, :], wt[:])

            if phase_c:
                tc.For_i_unrolled(0, OUTER, 1, body_c, max_unroll=uc)

    nc.compile()
    return nc, p


_CACHE = {}


def _get_full_kernel():
    if "full" not in _CACHE:
        _CACHE["full"] = build_kernel()
    return _CACHE["full"]


def kernel(vertex_attr, edge_attr, edgeij_pair):
    from concourse.bass_utils import run_bass_kernel_spmd

    nc, p = _get_full_kernel()
    NPAD = p["NPAD"]
    E_CORE = p["E_CORE"]
    U = p["U"]

    vertex_attr = np.asarray(vertex_attr, dtype=np.float32)
    edge_attr = np.ascontiguousarray(np.asarray(edge_attr, dtype=np.float32))
    srcf = np.ascontiguousarray(np.asarray(edgeij_pair, dtype=np.int32)[0])

    vpad = np.ones((NPAD, 2), dtype=np.float32)
    vpad[:vertex_attr.shape[0]] = vertex_attr

    in_maps = []
    for c in range(N_CORES):
        sl = slice(c * E_CORE, (c + 1) * E_CORE)
        outer = E_CORE // (P * U)
        in_maps.append({
            "src": srcf[sl].reshape(outer, P, U),
            "attr": edge_attr[sl].reshape(outer, P, 3 * U),
            "vattr": vpad,
        })
    res = run_bass_kernel_spmd(nc, in_maps, list(range(N_CORES)))
    return np.concatenate(
        [res.results[c]["w"].reshape(-1) for c in range(N_CORES)])
